# revision 14
# baseline (speedup 1.0000x reference)
"""CRF loss kernel for Trainium2 (8 NeuronCores, batch-sharded).

v2: twin-chain, alternating-layout forward algorithm in exp space.

Two independent recursions run concurrently per core and meet at a seam:
  fwd:  vf_l = Etilde_l^T vf_{l-1},  l = 1..255       (vf_0 from trellis[0])
  bwd:  wb_l = Etilde_l  wb_{l+1},   l = 511..256     (wb_512 = e_END)
  Z_b  = sum_j vf_255[j] * wb_256[j]
This halves the serial chain length and lets the two chains' engine ops
fill each other's dependency gaps.

Each chain processes a PAIR of steps per round with alternating layouts:
  A-step (SBUF v): e5[(g,jf),(jt,t)] = Ee * v[(g,t),jf]        (DVE TT)
                   psum = B1^T @ e5   (block-ones, CONSTANT bf16 stationary;
                   group-sums over jf land replicated over partitions) (PE)
  B-step (PSUM v): e6[(g,jt),(t,jf)] = Ee^T_layout * psum      (DVE TT)
                   v[(g,jt),t] = sum_jf e6                     (DVE reduce)
The PSUM replication trick means the B-step reads the matmul output
elementwise -- no diagonal pick, no D3 mask multiply. 3 DVE ops + 1 bf16
matmul per 2 steps vs 6 DVE + 2 fp32 matmuls in v1.

All SBUF data is bf16 (halves DMA; enables DVE 2x modes); PSUM f32.
Host folds chem cost, mask->identity and the e^{-kappa} rescale into the
shipped log-trellis; gold energy and the final log/seam-dot are host work.
"""
import numpy as np
import ml_dtypes

import concourse.bass as bass
import concourse.mybir as mybir
from concourse.bass_utils import run_bass_kernel_spmd

T = 20
START, END = 17, 18
KAPPA = 3.7881286
L, B = 512, 128
NCORES = 8
BS = B // NCORES
NEG = -100.0
F32 = mybir.dt.float32
BF16 = mybir.dt.bfloat16
GROUPS = [6, 6, 4]
PROFILE_DIR = None
LAST_RESULT = None

LF = 255          # fwd steps l = 1..255
LB_ = 256         # bwd steps l = 511..256
PF = LF * 60      # fwd pitch (elements per partition)
PB = LB_ * 60
CH = 32           # exp/DMA chunk (steps)
NCF = (LF + CH - 1) // CH   # 8 (last chunk 31 steps)
NCB = LB_ // CH             # 8


def _build_bass():
    nc = bass.Bass("TRN2", num_devices=NCORES, detect_race_conditions=False)
    AF_d = nc.declare_dram_parameter("AF", [120, PF], BF16, isOutput=False)
    AB_d = nc.declare_dram_parameter("AB", [120, PB], BF16, isOutput=False)
    v0F_d = nc.declare_dram_parameter("v0F", [120, 3], BF16, isOutput=False)
    v0B_d = nc.declare_dram_parameter("v0B", [120, 3], BF16, isOutput=False)
    B1_d = nc.declare_dram_parameter("B1", [120, 120], BF16, isOutput=False)
    outF_d = nc.declare_dram_parameter("outF", [120, 60], F32, isOutput=True)
    outB_d = nc.declare_dram_parameter("outB", [120, 3], F32, isOutput=True)

    add = mybir.AluOpType.add
    import contextlib

    es = contextlib.ExitStack()
    with es:
        rawF = es.enter_context(nc.sbuf_tensor("rawF", [120, PF], BF16))
        rawB = es.enter_context(nc.sbuf_tensor("rawB", [120, PB], BF16))
        EeF = es.enter_context(nc.sbuf_tensor("EeF", [120, PF], BF16))
        EeB = es.enter_context(nc.sbuf_tensor("EeB", [120, PB], BF16))
        e5F = es.enter_context(nc.sbuf_tensor("e5F", [120, 60], BF16))
        e6F = es.enter_context(nc.sbuf_tensor("e6F", [120, 60], BF16))
        e5B = es.enter_context(nc.sbuf_tensor("e5B", [120, 60], BF16))
        e6B = es.enter_context(nc.sbuf_tensor("e6B", [120, 60], BF16))
        vF = es.enter_context(nc.sbuf_tensor("vF", [120, 3], BF16))
        vB = es.enter_context(nc.sbuf_tensor("vB", [120, 3], BF16))
        B1_sb = es.enter_context(nc.sbuf_tensor("B1_sb", [120, 120], BF16))
        outF_sb = es.enter_context(nc.sbuf_tensor("outF_sb", [120, 60], F32))
        outB_sb = es.enter_context(nc.sbuf_tensor("outB_sb", [120, 3], F32))
        psF = [es.enter_context(nc.psum_tensor(f"psF{i}", [120, 60], F32)) for i in range(2)]
        psB = [es.enter_context(nc.psum_tensor(f"psB{i}", [120, 60], F32)) for i in range(2)]

        s_dma = es.enter_context(nc.semaphore("s_dma"))
        s_expF = es.enter_context(nc.semaphore("s_expF"))
        s_expB = es.enter_context(nc.semaphore("s_expB"))
        s_e = es.enter_context(nc.semaphore("s_e"))
        s_mm = es.enter_context(nc.semaphore("s_mm"))
        s_v = es.enter_context(nc.semaphore("s_v"))
        block = es.enter_context(nc.Block())

        def fcols(c, n):  # chunk column range
            lo = c * CH * 60
            hi = min(n * 60, (c + 1) * CH * 60)
            return lo, hi

        @block.sync
        def _(sync):
            sync.dma_start(B1_sb[:, :], B1_d[:, :]).then_inc(s_dma, 16)
            sync.dma_start(vF[:, :], v0F_d[:, :]).then_inc(s_dma, 16)
            sync.dma_start(vB[:, :], v0B_d[:, :]).then_inc(s_dma, 16)
            for c in range(NCF):
                lo, hi = fcols(c, LF)
                sync.dma_start(rawF[:, lo:hi], AF_d[:, lo:hi]).then_inc(s_dma, 16)
                lo, hi = fcols(c, LB_)
                sync.dma_start(rawB[:, lo:hi], AB_d[:, lo:hi]).then_inc(s_dma, 16)
            sync.wait_ge(s_v, 2)
            sync.dma_start(outF_d[:, :], outF_sb[:, :]).then_inc(s_dma, 16)
            sync.dma_start(outB_d[:, :], outB_sb[:, :]).then_inc(s_dma, 16)

        @block.scalar
        def _(scalar):
            for c in range(NCF):
                scalar.wait_ge(s_dma, 48 + 16 * (2 * c + 1))
                lo, hi = fcols(c, LF)
                scalar.activation(
                    bass.AP(EeF, lo, [[PF, 120], [1, hi - lo]]),
                    bass.AP(rawF, lo, [[PF, 120], [1, hi - lo]]),
                    mybir.ActivationFunctionType.Exp,
                ).then_inc(s_expF, 1)
                scalar.wait_ge(s_dma, 48 + 16 * (2 * c + 2))
                lo, hi = fcols(c, LB_)
                scalar.activation(
                    bass.AP(EeB, lo, [[PB, 120], [1, hi - lo]]),
                    bass.AP(rawB, lo, [[PB, 120], [1, hi - lo]]),
                    mybir.ActivationFunctionType.Exp,
                ).then_inc(s_expB, 1)
            # fwd tail: copy final psum (vf_255 replicated) to SBUF f32
            scalar.wait_ge(s_mm, 255)
            scalar.activation(
                outF_sb[:, :], psF[1][:, :], mybir.ActivationFunctionType.Copy
            ).then_inc(s_v, 1)

        @block.tensor
        def _(tensor):
            tensor.wait_ge(s_dma, 48)
            for p in range(128):
                tensor.wait_ge(s_e, 2 * p + 1)
                tensor.matmul(
                    psF[p % 2][:, :], B1_sb[:, :],
                    bass.AP(e5F, 0, [[60, 120], [1, 60]]),
                    start=True, stop=True,
                ).then_inc(s_mm, 1)
                tensor.wait_ge(s_e, 2 * p + 2)
                tensor.matmul(
                    psB[p % 2][:, :], B1_sb[:, :],
                    bass.AP(e5B, 0, [[60, 120], [1, 60]]),
                    start=True, stop=True,
                ).then_inc(s_mm, 1)

        @block.vector
        def _(vector):
            curF = [0]
            curB = [0]

            def needF(idx):
                c = idx // CH + 1
                if c > curF[0]:
                    vector.wait_ge(s_expF, c)
                    curF[0] = c

            def needB(idx):
                c = idx // CH + 1
                if c > curB[0]:
                    vector.wait_ge(s_expB, c)
                    curB[0] = c

            def att(Ee, pitch, idx, v, e5):  # A-step multiply
                vector.tensor_mul(
                    bass.AP(e5, 0, [[60, 120], [3, 20], [1, 3]]),
                    bass.AP(Ee, idx * 60, [[pitch, 120], [3, 20], [1, 3]]),
                    bass.AP(v, 0, [[3, 120], [0, 20], [1, 3]]),
                ).then_inc(s_e, 1)

            def btt(Ee, pitch, idx, ps, e6):  # B-step multiply (reads PSUM)
                vector.tensor_mul(
                    bass.AP(e6, 0, [[60, 120], [20, 3], [1, 20]]),
                    bass.AP(Ee, idx * 60, [[pitch, 120], [20, 3], [1, 20]]),
                    bass.AP(ps, 0, [[60, 120], [1, 3], [3, 20]]),
                )

            def tr(e6, vout):  # reduce over jfrom
                vector.tensor_reduce(
                    bass.AP(vout, 0, [[3, 120], [1, 3]]),
                    bass.AP(e6, 0, [[60, 120], [20, 3], [1, 20]]),
                    axis=mybir.AxisListType.X, op=add,
                )

            vector.wait_ge(s_dma, 48)
            with nc.allow_low_precision("bf16 recurrence validated vs numpy sim"):
                needF(0)
                att(EeF, PF, 0, vF, e5F)
                needB(0)
                att(EeB, PB, 0, vB, e5B)
                for p in range(127):
                    needF(2 * p + 2)
                    needB(2 * p + 2)
                    # one wait covers both chains' psum reads: PE (900ns/round)
                    # runs ahead of DVE (~1340ns/round), so MM_B(p) is done by
                    # the time DVE reaches here in steady state
                    vector.wait_ge(s_mm, 2 * p + 2)
                    btt(EeF, PF, 2 * p + 1, psF[p % 2], e6F)
                    tr(e6F, vF)
                    att(EeF, PF, 2 * p + 2, vF, e5F)
                    btt(EeB, PB, 2 * p + 1, psB[p % 2], e6B)
                    tr(e6B, vB)
                    att(EeB, PB, 2 * p + 2, vB, e5B)
                # bwd tail: pair 127 B-step, reduce straight to f32 output
                vector.wait_ge(s_mm, 256)
                btt(EeB, PB, 255, psB[1], e6B)
                vector.tensor_reduce(
                    bass.AP(outB_sb, 0, [[3, 120], [1, 3]]),
                    bass.AP(e6B, 0, [[60, 120], [20, 3], [1, 20]]),
                    axis=mybir.AxisListType.X, op=add,
                ).then_inc(s_v, 1)

    return nc


_NC_CACHE = None


def _get_nc():
    global _NC_CACHE
    if _NC_CACHE is None:
        _NC_CACHE = _build_bass()
    return _NC_CACHE


def kernel(trellis, gold, mask, corpus_mask):
    trellis = np.asarray(trellis, dtype=np.float32)
    gold = np.asarray(gold).astype(np.int64)
    mask = np.asarray(mask).astype(bool)
    bf = ml_dtypes.bfloat16

    # ---- host: gold energy + per-b unmasked counts ----
    flat = trellis.reshape(L, B, T * T)
    energy = np.take_along_axis(flat, gold, axis=2)[..., 0]
    gold_energy = float(np.sum(np.where(mask, energy.astype(np.float64), 0.0)))
    cnt = mask[1:].sum(axis=0).astype(np.float64)

    # ---- host: fold chem cost, mask->identity, -kappa ----
    chem = np.zeros((T,), np.float32)
    chem[:4] = 1.0
    lnI = np.full((T, T), NEG, np.float32)
    np.fill_diagonal(lnI, 0.0)
    Tt = trellis + (chem - np.float32(KAPPA))[None, None, None, :]
    Tt = np.where(mask[:, :, None, None], Tt, lnI[None, None])

    lsA_f = np.arange(1, 256, 2)        # fwd A-steps (128)
    lsB_f = np.arange(2, 255, 2)        # fwd B-steps (127)
    lsA_b = 511 - np.arange(0, 256, 2)  # bwd A-steps: 511,509,..,257 (128)
    lsB_b = 511 - np.arange(1, 256, 2)  # bwd B-steps: 510,508,..,256 (128)
    TtAf, TtBf = Tt[lsA_f], Tt[lsB_f]
    TtAb, TtBb = Tt[lsA_b], Tt[lsB_b]

    B1 = np.zeros((120, 120), np.float32)
    for k in range(120):
        B1[k, (k // T) * T:(k // T + 1) * T] = 1.0
    B1 = B1.astype(bf)

    def slots(cidx):
        out = []
        for t in range(3):
            for g in range(6):
                if g < GROUPS[t]:
                    b_loc = (6 * t + g) if t < 2 else (12 + g)
                    out.append((g, t, cidx * BS + b_loc))
                else:
                    out.append((g, t, None))
        return out

    in_maps = []
    for cidx in range(NCORES):
        AF3 = np.empty((120, LF, 60), np.float32)
        AB3 = np.empty((120, LB_, 60), np.float32)
        v0F = np.zeros((120, 3), np.float32)
        v0B = np.zeros((120, 3), np.float32)
        for (g, t, b) in slots(cidx):
            rows = slice(20 * g, 20 * g + 20)
            evF = AF3[rows, 0::2, :].reshape(20, 128, 20, 3)
            odF = AF3[rows, 1::2, :].reshape(20, 127, 3, 20)
            evB = AB3[rows, 0::2, :].reshape(20, 128, 20, 3)
            odB = AB3[rows, 1::2, :].reshape(20, 128, 3, 20)
            if b is not None:
                evF[:, :, :, t] = TtAf[:, b].transpose(1, 0, 2)
                odF[:, :, t, :] = TtBf[:, b].transpose(2, 0, 1)
                evB[:, :, :, t] = TtAb[:, b].transpose(2, 0, 1)
                odB[:, :, t, :] = TtBb[:, b].transpose(1, 0, 2)
                v0F[rows, t] = np.exp(trellis[0, b, START, :])
            else:
                evF[:, :, :, t] = lnI[:, None, :]
                odF[:, :, t, :] = lnI[:, None, :]
                evB[:, :, :, t] = lnI[:, None, :]
                odB[:, :, t, :] = lnI[:, None, :]
                v0F[20 * g + START, t] = 1.0
            v0B[20 * g + END, t] = 1.0
        in_maps.append({
            "AF": AF3.reshape(120, PF).astype(bf),
            "AB": AB3.reshape(120, PB).astype(bf),
            "v0F": v0F.astype(bf), "v0B": v0B.astype(bf), "B1": B1,
        })

    nc = _get_nc()
    global LAST_RESULT
    kw = dict(trace=True, tmpdir=PROFILE_DIR) if PROFILE_DIR else {}
    LAST_RESULT = run_bass_kernel_spmd(nc, in_maps, list(range(NCORES)), **kw)
    res = LAST_RESULT.results

    partition = 0.0
    for cidx in range(NCORES):
        oF = np.asarray(res[cidx]["outF"], dtype=np.float64)
        oB = np.asarray(res[cidx]["outB"], dtype=np.float64)
        for (g, t, b) in slots(cidx):
            if b is None:
                continue
            vf = oF[20 * g, t::3]            # [20] = vf_255[j]
            wb = oB[20 * g:20 * g + 20, t]   # [20] = wb_256[j]
            partition += np.log(np.dot(vf, wb)) + KAPPA * cnt[b]
    return np.float32((partition - gold_energy) / B)


# revision 15
# speedup vs baseline: 1.2954x; 1.2954x over previous
"""CRF loss kernel for Trainium2 (8 NeuronCores, batch-sharded).

v2: twin-chain, alternating-layout forward algorithm in exp space.

Two independent recursions run concurrently per core and meet at a seam:
  fwd:  vf_l = Etilde_l^T vf_{l-1},  l = 1..255       (vf_0 from trellis[0])
  bwd:  wb_l = Etilde_l  wb_{l+1},   l = 511..256     (wb_512 = e_END)
  Z_b  = sum_j vf_255[j] * wb_256[j]
This halves the serial chain length and lets the two chains' engine ops
fill each other's dependency gaps.

Each chain processes a PAIR of steps per round with alternating layouts:
  A-step (SBUF v): e5[(g,jf),(jt,t)] = Ee * v[(g,t),jf]        (DVE TT)
                   psum = B1^T @ e5   (block-ones, CONSTANT bf16 stationary;
                   group-sums over jf land replicated over partitions) (PE)
  B-step (PSUM v): e6[(g,jt),(t,jf)] = Ee^T_layout * psum      (DVE TT)
                   v[(g,jt),t] = sum_jf e6                     (DVE reduce)
The PSUM replication trick means the B-step reads the matmul output
elementwise -- no diagonal pick, no D3 mask multiply. 3 DVE ops + 1 bf16
matmul per 2 steps vs 6 DVE + 2 fp32 matmuls in v1.

All SBUF data is bf16 (halves DMA; enables DVE 2x modes); PSUM f32.
Host folds chem cost, mask->identity and the e^{-kappa} rescale into the
shipped log-trellis; gold energy and the final log/seam-dot are host work.
"""
import numpy as np
import ml_dtypes

import concourse.bass as bass
import concourse.mybir as mybir
from concourse.bass_utils import run_bass_kernel_spmd

T = 20
START, END = 17, 18
KAPPA = 3.7881286
L, B = 512, 128
NCORES = 8
BS = B // NCORES
NEG = -100.0
F32 = mybir.dt.float32
BF16 = mybir.dt.bfloat16
GROUPS = [6, 6, 4]
PROFILE_DIR = None
LAST_RESULT = None

LF = 255          # fwd steps l = 1..255
LB_ = 256         # bwd steps l = 511..256
PF = LF * 60      # fwd pitch (elements per partition)
PB = LB_ * 60
CH = 32           # exp/DMA chunk (steps)
NCF = (LF + CH - 1) // CH   # 8 (last chunk 31 steps)
NCB = LB_ // CH             # 8


def _build_bass():
    nc = bass.Bass("TRN2", num_devices=NCORES, detect_race_conditions=False)
    AF_d = nc.declare_dram_parameter("AF", [120, PF], BF16, isOutput=False)
    AB_d = nc.declare_dram_parameter("AB", [120, PB], BF16, isOutput=False)
    v0F_d = nc.declare_dram_parameter("v0F", [120, 3], BF16, isOutput=False)
    v0B_d = nc.declare_dram_parameter("v0B", [120, 3], BF16, isOutput=False)
    B1_d = nc.declare_dram_parameter("B1", [120, 120], BF16, isOutput=False)
    outF_d = nc.declare_dram_parameter("outF", [120, 60], F32, isOutput=True)
    outB_d = nc.declare_dram_parameter("outB", [120, 3], F32, isOutput=True)

    add = mybir.AluOpType.add
    import contextlib

    es = contextlib.ExitStack()
    with es:
        rawF = es.enter_context(nc.sbuf_tensor("rawF", [120, PF], BF16))
        rawB = es.enter_context(nc.sbuf_tensor("rawB", [120, PB], BF16))
        EeF = es.enter_context(nc.sbuf_tensor("EeF", [120, PF], BF16))
        EeB = es.enter_context(nc.sbuf_tensor("EeB", [120, PB], BF16))
        e5F = es.enter_context(nc.sbuf_tensor("e5F", [120, 60], BF16))
        e6F = es.enter_context(nc.sbuf_tensor("e6F", [120, 60], BF16))
        e5B = es.enter_context(nc.sbuf_tensor("e5B", [120, 60], BF16))
        e6B = es.enter_context(nc.sbuf_tensor("e6B", [120, 60], BF16))
        vF = es.enter_context(nc.sbuf_tensor("vF", [120, 3], BF16))
        vB = es.enter_context(nc.sbuf_tensor("vB", [120, 3], BF16))
        B1_sb = es.enter_context(nc.sbuf_tensor("B1_sb", [120, 120], BF16))
        outF_sb = es.enter_context(nc.sbuf_tensor("outF_sb", [120, 60], F32))
        outB_sb = es.enter_context(nc.sbuf_tensor("outB_sb", [120, 3], F32))
        psF = [es.enter_context(nc.psum_tensor(f"psF{i}", [120, 60], F32)) for i in range(2)]
        psB = [es.enter_context(nc.psum_tensor(f"psB{i}", [120, 60], F32)) for i in range(2)]

        s_dma = es.enter_context(nc.semaphore("s_dma"))
        s_expF = es.enter_context(nc.semaphore("s_expF"))
        s_expB = es.enter_context(nc.semaphore("s_expB"))
        s_e = es.enter_context(nc.semaphore("s_e"))
        s_mm = es.enter_context(nc.semaphore("s_mm"))
        s_v = es.enter_context(nc.semaphore("s_v"))
        block = es.enter_context(nc.Block())

        def fcols(c, n):  # chunk column range
            lo = c * CH * 60
            hi = min(n * 60, (c + 1) * CH * 60)
            return lo, hi

        @block.sync
        def _(sync):
            sync.dma_start(B1_sb[:, :], B1_d[:, :]).then_inc(s_dma, 16)
            sync.dma_start(vF[:, :], v0F_d[:, :]).then_inc(s_dma, 16)
            sync.dma_start(vB[:, :], v0B_d[:, :]).then_inc(s_dma, 16)
            for c in range(NCF):
                lo, hi = fcols(c, LF)
                sync.dma_start(rawF[:, lo:hi], AF_d[:, lo:hi]).then_inc(s_dma, 16)
                lo, hi = fcols(c, LB_)
                sync.dma_start(rawB[:, lo:hi], AB_d[:, lo:hi]).then_inc(s_dma, 16)
            sync.wait_ge(s_v, 2)
            sync.dma_start(outF_d[:, :], outF_sb[:, :]).then_inc(s_dma, 16)
            sync.dma_start(outB_d[:, :], outB_sb[:, :]).then_inc(s_dma, 16)

        @block.scalar
        def _(scalar):
            for c in range(NCF):
                scalar.wait_ge(s_dma, 48 + 16 * (2 * c + 1))
                lo, hi = fcols(c, LF)
                scalar.activation(
                    bass.AP(EeF, lo, [[PF, 120], [1, hi - lo]]),
                    bass.AP(rawF, lo, [[PF, 120], [1, hi - lo]]),
                    mybir.ActivationFunctionType.Exp,
                ).then_inc(s_expF, 1)
                scalar.wait_ge(s_dma, 48 + 16 * (2 * c + 2))
                lo, hi = fcols(c, LB_)
                scalar.activation(
                    bass.AP(EeB, lo, [[PB, 120], [1, hi - lo]]),
                    bass.AP(rawB, lo, [[PB, 120], [1, hi - lo]]),
                    mybir.ActivationFunctionType.Exp,
                ).then_inc(s_expB, 1)
            # fwd tail: copy final psum (vf_255 replicated) to SBUF f32
            scalar.wait_ge(s_mm, 255)
            scalar.activation(
                outF_sb[:, :], psF[1][:, :], mybir.ActivationFunctionType.Copy
            ).then_inc(s_v, 1)

        @block.tensor
        def _(tensor):
            tensor.wait_ge(s_dma, 48)
            for p in range(128):
                tensor.wait_ge(s_e, 2 * p + 1)
                tensor.matmul(
                    psF[p % 2][:, :], B1_sb[:, :],
                    bass.AP(e5F, 0, [[60, 120], [1, 60]]),
                    start=True, stop=True,
                ).then_inc(s_mm, 1)
                tensor.wait_ge(s_e, 2 * p + 2)
                tensor.matmul(
                    psB[p % 2][:, :], B1_sb[:, :],
                    bass.AP(e5B, 0, [[60, 120], [1, 60]]),
                    start=True, stop=True,
                ).then_inc(s_mm, 1)

        @block.vector
        def _(vector):
            curF = [0]
            curB = [0]

            def needF(idx):
                c = idx // CH + 1
                if c > curF[0]:
                    vector.wait_ge(s_expF, c)
                    curF[0] = c

            def needB(idx):
                c = idx // CH + 1
                if c > curB[0]:
                    vector.wait_ge(s_expB, c)
                    curB[0] = c

            def att(Ee, pitch, idx, v, e5):  # A-step multiply
                vector.tensor_mul(
                    bass.AP(e5, 0, [[60, 120], [3, 20], [1, 3]]),
                    bass.AP(Ee, idx * 60, [[pitch, 120], [3, 20], [1, 3]]),
                    bass.AP(v, 0, [[3, 120], [0, 20], [1, 3]]),
                ).then_inc(s_e, 1)

            def btt(Ee, pitch, idx, ps, e6):  # B-step multiply (reads PSUM)
                vector.tensor_mul(
                    bass.AP(e6, 0, [[60, 120], [20, 3], [1, 20]]),
                    bass.AP(Ee, idx * 60, [[pitch, 120], [20, 3], [1, 20]]),
                    bass.AP(ps, 0, [[60, 120], [1, 3], [3, 20]]),
                )

            def tr(e6, vout):  # reduce over jfrom
                vector.tensor_reduce(
                    bass.AP(vout, 0, [[3, 120], [1, 3]]),
                    bass.AP(e6, 0, [[60, 120], [20, 3], [1, 20]]),
                    axis=mybir.AxisListType.X, op=add,
                )

            vector.wait_ge(s_dma, 48)
            with nc.allow_low_precision("bf16 recurrence validated vs numpy sim"):
                needF(0)
                att(EeF, PF, 0, vF, e5F)
                needB(0)
                att(EeB, PB, 0, vB, e5B)
                for p in range(127):
                    needF(2 * p + 2)
                    vector.wait_ge(s_mm, 2 * p + 1)
                    btt(EeF, PF, 2 * p + 1, psF[p % 2], e6F)
                    tr(e6F, vF)
                    att(EeF, PF, 2 * p + 2, vF, e5F)
                    needB(2 * p + 2)
                    vector.wait_ge(s_mm, 2 * p + 2)
                    btt(EeB, PB, 2 * p + 1, psB[p % 2], e6B)
                    tr(e6B, vB)
                    att(EeB, PB, 2 * p + 2, vB, e5B)
                # bwd tail: pair 127 B-step, reduce straight to f32 output
                vector.wait_ge(s_mm, 256)
                btt(EeB, PB, 255, psB[1], e6B)
                vector.tensor_reduce(
                    bass.AP(outB_sb, 0, [[3, 120], [1, 3]]),
                    bass.AP(e6B, 0, [[60, 120], [20, 3], [1, 20]]),
                    axis=mybir.AxisListType.X, op=add,
                ).then_inc(s_v, 1)

    return nc


_NC_CACHE = None


def _get_nc():
    global _NC_CACHE
    if _NC_CACHE is None:
        _NC_CACHE = _build_bass()
    return _NC_CACHE


def kernel(trellis, gold, mask, corpus_mask):
    trellis = np.asarray(trellis, dtype=np.float32)
    gold = np.asarray(gold).astype(np.int64)
    mask = np.asarray(mask).astype(bool)
    bf = ml_dtypes.bfloat16

    # ---- host: gold energy + per-b unmasked counts ----
    flat = trellis.reshape(L, B, T * T)
    energy = np.take_along_axis(flat, gold, axis=2)[..., 0]
    gold_energy = float(np.sum(np.where(mask, energy.astype(np.float64), 0.0)))
    cnt = mask[1:].sum(axis=0).astype(np.float64)

    # ---- host: fold chem cost, mask->identity, -kappa ----
    chem = np.zeros((T,), np.float32)
    chem[:4] = 1.0
    lnI = np.full((T, T), NEG, np.float32)
    np.fill_diagonal(lnI, 0.0)
    Tt = trellis + (chem - np.float32(KAPPA))[None, None, None, :]
    Tt = np.where(mask[:, :, None, None], Tt, lnI[None, None])

    lsA_f = np.arange(1, 256, 2)        # fwd A-steps (128)
    lsB_f = np.arange(2, 255, 2)        # fwd B-steps (127)
    lsA_b = 511 - np.arange(0, 256, 2)  # bwd A-steps: 511,509,..,257 (128)
    lsB_b = 511 - np.arange(1, 256, 2)  # bwd B-steps: 510,508,..,256 (128)
    TtAf, TtBf = Tt[lsA_f], Tt[lsB_f]
    TtAb, TtBb = Tt[lsA_b], Tt[lsB_b]

    B1 = np.zeros((120, 120), np.float32)
    for k in range(120):
        B1[k, (k // T) * T:(k // T + 1) * T] = 1.0
    B1 = B1.astype(bf)

    def slots(cidx):
        out = []
        for t in range(3):
            for g in range(6):
                if g < GROUPS[t]:
                    b_loc = (6 * t + g) if t < 2 else (12 + g)
                    out.append((g, t, cidx * BS + b_loc))
                else:
                    out.append((g, t, None))
        return out

    in_maps = []
    for cidx in range(NCORES):
        AF3 = np.empty((120, LF, 60), np.float32)
        AB3 = np.empty((120, LB_, 60), np.float32)
        v0F = np.zeros((120, 3), np.float32)
        v0B = np.zeros((120, 3), np.float32)
        for (g, t, b) in slots(cidx):
            rows = slice(20 * g, 20 * g + 20)
            evF = AF3[rows, 0::2, :].reshape(20, 128, 20, 3)
            odF = AF3[rows, 1::2, :].reshape(20, 127, 3, 20)
            evB = AB3[rows, 0::2, :].reshape(20, 128, 20, 3)
            odB = AB3[rows, 1::2, :].reshape(20, 128, 3, 20)
            if b is not None:
                evF[:, :, :, t] = TtAf[:, b].transpose(1, 0, 2)
                odF[:, :, t, :] = TtBf[:, b].transpose(2, 0, 1)
                evB[:, :, :, t] = TtAb[:, b].transpose(2, 0, 1)
                odB[:, :, t, :] = TtBb[:, b].transpose(1, 0, 2)
                v0F[rows, t] = np.exp(trellis[0, b, START, :])
            else:
                evF[:, :, :, t] = lnI[:, None, :]
                odF[:, :, t, :] = lnI[:, None, :]
                evB[:, :, :, t] = lnI[:, None, :]
                odB[:, :, t, :] = lnI[:, None, :]
                v0F[20 * g + START, t] = 1.0
            v0B[20 * g + END, t] = 1.0
        in_maps.append({
            "AF": AF3.reshape(120, PF).astype(bf),
            "AB": AB3.reshape(120, PB).astype(bf),
            "v0F": v0F.astype(bf), "v0B": v0B.astype(bf), "B1": B1,
        })

    nc = _get_nc()
    global LAST_RESULT
    kw = dict(trace=True, tmpdir=PROFILE_DIR) if PROFILE_DIR else {}
    LAST_RESULT = run_bass_kernel_spmd(nc, in_maps, list(range(NCORES)), **kw)
    res = LAST_RESULT.results

    partition = 0.0
    for cidx in range(NCORES):
        oF = np.asarray(res[cidx]["outF"], dtype=np.float64)
        oB = np.asarray(res[cidx]["outB"], dtype=np.float64)
        for (g, t, b) in slots(cidx):
            if b is None:
                continue
            vf = oF[20 * g, t::3]            # [20] = vf_255[j]
            wb = oB[20 * g:20 * g + 20, t]   # [20] = wb_256[j]
            partition += np.log(np.dot(vf, wb)) + KAPPA * cnt[b]
    return np.float32((partition - gold_energy) / B)
